# revision 4
# baseline (speedup 1.0000x reference)
"""DiagonalElmanCell Trainium2 kernel.

reference:
    alpha = sigmoid(alpha_raw)                      # [D]
    x_proj = einsum('tbd,ed->tbe', x, W_x) + b      # [T,B,D]
    gate = silu(x + b_gate)                         # [T,B,D]
    h_t = tanh(x_proj[t] + alpha * h_{t-1})         # scan over T
    output = hs * gate ; h = concat([h0[None], hs])

Strategy: batch-shard B=16 over 8 cores (2 per core). Per core:
  phase 1: xp = x @ W_x.T + b via PE (float32r), gate = silu(x+b_gate) on ACT,
           in a d-on-partition transposed layout (host pre-transposes x).
  phase 2: the T-recurrence is chunked into C chunks of length L processed in
           lockstep as wide [128, eo*C*b] tiles. Each chunk (except the first)
           starts W steps early from h=0; because |d h_t / d h_{t-1}| <= alpha
           < 1 the warmup state converges to the true trajectory at rate
           alpha^W (W chosen from max(alpha) so the error is < ~1e-9, far
           below fp32 noise). Chunk 0 is seeded exactly with h0.
Outputs are written in a device-friendly layout and un-permuted on host.
"""

import math
import os
import sys

sys.path.insert(0, "/opt/trn_rl_repo")

import numpy as np

import concourse.bacc as bacc
import concourse.mybir as mybir
from concourse.tile import TileContext
from concourse.bass_utils import run_bass_kernel_spmd

T, B, D = 1024, 16, 1024
NCORES = 8
BC = B // NCORES          # batch per core
EO = D // 128             # e_o blocks (free-dim replication of d)
F32 = mybir.dt.float32
F32R = mybir.dt.float32r
AF = mybir.ActivationFunctionType


def _pick_schedule(alpha_max: float):
    """Choose (C, L, W): C chunks of length L, W warmup steps."""
    # error after W warmup steps <= alpha_max^W ; want < 1e-9
    if alpha_max >= 1.0 - 1e-9:
        return 1, T, 0
    w_req = int(math.ceil(math.log(1e-9) / math.log(alpha_max)))
    w_req = max(w_req, 4)
    for C in (32, 16, 8, 4, 2):
        L = T // C
        if w_req <= L:
            return C, L, min(max(w_req, 12), L)
    return 1, T, 0


def _build(C: int, L: int, W: int, nt: int, groups: int):
    """Build the per-core Bass program (SPMD: same program on all cores)."""
    nc = bacc.Bacc(target_bir_lowering=False)

    xt = nc.dram_tensor("xt", [D, T, BC], F32R, kind="ExternalInput")
    wT = nc.dram_tensor("wT", [D, D], F32R, kind="ExternalInput")
    alpha_rec = nc.dram_tensor("alpha_rec", [128, EO * C * BC], F32, kind="ExternalInput")
    b_ev = nc.dram_tensor("b_ev", [128, EO], F32, kind="ExternalInput")
    bg_ev = nc.dram_tensor("bg_ev", [128, EO], F32, kind="ExternalInput")
    h0t = nc.dram_tensor("h0t", [128, EO * BC], F32, kind="ExternalInput")
    outT = nc.dram_tensor("outT", [L, 128, EO, C, BC], F32, kind="ExternalOutput")
    hT = nc.dram_tensor("hT", [L, 128, EO, C, BC], F32, kind="ExternalOutput")

    TP = T + W                # padded t-extent of the xp buffer (front pad W)
    NB = T // nt              # matmul n-blocks (nt t-values each)
    N = nt * BC               # moving cols per matmul
    steps = W + L
    eg = EO // groups         # e_o blocks per group
    gw = eg * C * BC          # per-group state width

    with TileContext(nc) as tc:
        with (
            tc.tile_pool(name="big", bufs=1) as big,
            tc.tile_pool(name="stream", bufs=2) as stream,
            tc.tile_pool(name="small", bufs=1) as small,
            tc.tile_pool(name="state", bufs=1) as statep,
            tc.tile_pool(name="og", bufs=3) as ogp,
            tc.tile_pool(name="psum", bufs=8, space="PSUM") as psum,
        ):
            # resident buffers
            xp_sb = big.tile([128, EO * TP * BC], F32, tag="xp")
            gate_sb = big.tile([128, EO * T * BC], F32, tag="gate")
            w_sb = [big.tile([128, D], F32R, tag=f"w{k}", name=f"w{k}") for k in range(EO)]
            al_sb = small.tile([128, EO * C * BC], F32, tag="alpha")
            bev_sb = small.tile([128, EO], F32, tag="bev")
            bgev_sb = small.tile([128, EO], F32, tag="bgev")
            h0_sb = small.tile([128, EO * BC], F32, tag="h0")

            for k in range(EO):
                nc.sync.dma_start(out=w_sb[k], in_=wT[k * 128:(k + 1) * 128, :])
            nc.sync.dma_start(out=al_sb, in_=alpha_rec[:, :])
            nc.sync.dma_start(out=bev_sb, in_=b_ev[:, :])
            nc.sync.dma_start(out=bgev_sb, in_=bg_ev[:, :])
            nc.sync.dma_start(out=h0_sb, in_=h0t[:, :])

            xp4 = xp_sb.rearrange("p (e t b) -> p e t b", e=EO, b=BC)
            gate4 = gate_sb.rearrange("p (e t b) -> p e t b", e=EO, b=BC)

            # zero the warmup pad region [0, W) of xp
            if W > 0:
                nc.vector.memset(xp4[:, :, 0:W, :], 0.0)

            # ---------------- phase 1: matmul + gate ----------------
            for n in range(NB):
                t0 = n * nt
                xtiles = []
                for k in range(EO):
                    xt_t = stream.tile([128, N], F32R, tag=f"xt{k}")
                    nc.sync.dma_start(
                        out=xt_t.rearrange("p (t b) -> p t b", b=BC),
                        in_=xt[k * 128:(k + 1) * 128, t0:t0 + nt, :],
                    )
                    xtiles.append(xt_t)
                    # gate = silu(x + b_gate) in the same transposed layout
                    nc.scalar.activation(
                        gate4[:, k, t0:t0 + nt, :].rearrange("p t b -> p (t b)"),
                        xt_t.bitcast(F32),
                        AF.Silu,
                        bias=bgev_sb[:, k:k + 1],
                    )
                for em in range(EO):
                    ps = psum.tile([128, N], F32, tag="ps")
                    for k in range(EO):
                        nc.tensor.matmul(
                            ps,
                            w_sb[k][:, em * 128:(em + 1) * 128],
                            xtiles[k],
                            start=(k == 0),
                            stop=(k == EO - 1),
                        )
                    # evict with +b bias into the padded xp buffer
                    nc.scalar.activation(
                        xp4[:, em, W + t0:W + t0 + nt, :].rearrange("p t b -> p (t b)"),
                        ps,
                        AF.Identity,
                        bias=bev_sb[:, em:em + 1],
                    )

            # ---------------- phase 2: recurrence ----------------
            # state layout: [128, (e_o, c, b)]; group-split over e_o.
            # h is double-buffered so the hT DMA read never blocks the next
            # step's tanh write.
            h_t = [[statep.tile([128, gw], F32, tag=f"h{g}_{i}", name=f"h{g}_{i}") for i in range(2)]
                   for g in range(groups)]
            u_t = [statep.tile([128, gw], F32, tag=f"u{g}", name=f"u{g}") for g in range(groups)]

            for g in range(groups):
                nc.vector.memset(h_t[g][0], 0.0)
                nc.vector.memset(h_t[g][1], 0.0)

            al3 = al_sb.rearrange("p (e c b) -> p e c b", e=EO, b=BC)
            h03 = h0_sb.rearrange("p (e b) -> p e b", e=EO)

            def gath(buf4, s, g):
                # strided chunk-gather: t = c*L + s for c in [0, C)
                v = buf4[:, :, s::L, :]
                return v[:, g * eg:(g + 1) * eg, 0:C, :]

            for s in range(steps):
                hp = h_t_idx = (s + 1) % 2   # h_prev buffer index
                hn = s % 2                   # h_new buffer index
                if s == W:
                    # inject exact h0 into chunk 0 of every group (h_prev)
                    for g in range(groups):
                        hv = h_t[g][hp].rearrange(
                            "p (e c b) -> p e c b", e=eg, b=BC
                        )
                        nc.vector.tensor_copy(
                            hv[:, :, 0, :], h03[:, g * eg:(g + 1) * eg, :]
                        )
                for g in range(groups):
                    av = al3[:, g * eg:(g + 1) * eg, :, :]
                    u3 = u_t[g].rearrange("p (e c b) -> p e c b", e=eg, b=BC)
                    h3p = h_t[g][hp].rearrange("p (e c b) -> p e c b", e=eg, b=BC)
                    # u = h * alpha ; u += xp[t(s)] ; h = tanh(u)
                    nc.vector.tensor_mul(u3, h3p, av)
                    nc.vector.tensor_add(u3, u3, gath(xp4, s, g))
                    nc.scalar.activation(h_t[g][hn], u_t[g], AF.Tanh)
                if s >= W:
                    j = s - W
                    for g in range(groups):
                        h3n = h_t[g][hn].rearrange(
                            "p (e c b) -> p e c b", e=eg, b=BC
                        )
                        og = ogp.tile([128, gw], F32, tag=f"og{g}")
                        og3 = og.rearrange("p (e c b) -> p e c b", e=eg, b=BC)
                        nc.gpsimd.tensor_mul(og3, h3n, gath(gate4, j, g))
                        nc.sync.dma_start(
                            out=outT[j, :, g * eg:(g + 1) * eg, :, :],
                            in_=og3,
                        )
                        nc.sync.dma_start(
                            out=hT[j, :, g * eg:(g + 1) * eg, :, :],
                            in_=h3n,
                        )

    nc.finalize()
    return nc


_cache = {}


def _get_program(C, L, W, nt, groups):
    key = (C, L, W, nt, groups)
    if key not in _cache:
        _cache[key] = _build(C, L, W, nt, groups)
    return _cache[key]


def kernel(x, h0, W_x, alpha_raw, b, b_gate, _trace=False, _tmpdir=None):
    x = np.asarray(x, dtype=np.float32)
    h0 = np.asarray(h0, dtype=np.float32)
    W_x = np.asarray(W_x, dtype=np.float32)
    alpha_raw = np.asarray(alpha_raw, dtype=np.float32)
    b = np.asarray(b, dtype=np.float32)
    b_gate = np.asarray(b_gate, dtype=np.float32)

    alpha = (1.0 / (1.0 + np.exp(-alpha_raw.astype(np.float64)))).astype(np.float32)
    C, L, W = _pick_schedule(float(alpha.max()))
    nt = int(os.environ.get("K_NT", "128"))
    groups = int(os.environ.get("K_GROUPS", "2"))

    nc = _get_program(C, L, W, nt, groups)

    # host-side prep (not in HW time)
    xT = np.ascontiguousarray(x.transpose(2, 0, 1))          # [D, T, B]
    wT = np.ascontiguousarray(W_x.T)                          # [d, e]
    al_eo = np.ascontiguousarray(alpha.reshape(EO, 128).T)    # [128, EO]
    alpha_rec = np.ascontiguousarray(
        np.broadcast_to(al_eo[:, :, None, None], (128, EO, C, BC))
    ).reshape(128, EO * C * BC)
    b_ev = np.ascontiguousarray(b.reshape(EO, 128).T)
    bg_ev = np.ascontiguousarray(b_gate.reshape(EO, 128).T)

    in_maps = []
    for c8 in range(NCORES):
        bsl = slice(c8 * BC, (c8 + 1) * BC)
        h0c = h0[bsl, :]                                      # [BC, D]
        h0t = np.ascontiguousarray(
            h0c.T.reshape(EO, 128, BC).transpose(1, 0, 2)
        ).reshape(128, EO * BC)
        in_maps.append({
            "xt": np.ascontiguousarray(xT[:, :, bsl]),
            "wT": wT,
            "alpha_rec": alpha_rec,
            "b_ev": b_ev,
            "bg_ev": bg_ev,
            "h0t": h0t,
        })

    res = run_bass_kernel_spmd(
        nc, in_maps, core_ids=list(range(NCORES)),
        trace=_trace, tmpdir=_tmpdir,
    )

    out = np.empty((T, B, D), dtype=np.float32)
    h = np.empty((T + 1, B, D), dtype=np.float32)
    h[0] = h0
    for c8 in range(NCORES):
        bsl = slice(c8 * BC, (c8 + 1) * BC)
        r = res.results[c8]
        # [L(j), e_i, e_o, c, b] -> (c, j, b, e_o, e_i) -> [T, BC, D]
        out[:, bsl, :] = r["outT"].transpose(3, 0, 4, 2, 1).reshape(T, BC, D)
        h[1:, bsl, :] = r["hT"].transpose(3, 0, 4, 2, 1).reshape(T, BC, D)
    if _trace:
        kernel._last_result = res
    return out, h


# revision 10
# speedup vs baseline: 1.0575x; 1.0575x over previous
"""DiagonalElmanCell Trainium2 kernel.

reference:
    alpha = sigmoid(alpha_raw)                      # [D]
    x_proj = einsum('tbd,ed->tbe', x, W_x) + b      # [T,B,D]
    gate = silu(x + b_gate)                         # [T,B,D]
    h_t = tanh(x_proj[t] + alpha * h_{t-1})         # scan over T
    output = hs * gate ; h = concat([h0[None], hs])

Strategy: batch-shard B=16 over 8 cores (2 per core). Per core:
  phase 1: xp = x @ W_x.T + b via PE (float32r), gate = silu(x+b_gate) on ACT,
           in a d-on-partition transposed layout (host pre-transposes x).
  phase 2: the T-recurrence is chunked into C chunks of length L processed in
           lockstep as wide [128, eo*C*b] tiles. Each chunk (except the first)
           starts W steps early from h=0; because |d h_t / d h_{t-1}| <= alpha
           < 1 the warmup state converges to the true trajectory at rate
           alpha^W (W chosen from max(alpha) so the error is < ~1e-9, far
           below fp32 noise). Chunk 0 is seeded exactly with h0.
Outputs are written in a device-friendly layout and un-permuted on host.
"""

import math
import os
import sys

sys.path.insert(0, "/opt/trn_rl_repo")

import numpy as np

import concourse.bacc as bacc
import concourse.mybir as mybir
from concourse.tile import TileContext
from concourse.bass_utils import run_bass_kernel_spmd

T, B, D = 1024, 16, 1024
NCORES = 8
BC = B // NCORES          # batch per core
EO = D // 128             # e_o blocks (free-dim replication of d)
F32 = mybir.dt.float32
F32R = mybir.dt.float32r
AF = mybir.ActivationFunctionType


def _pick_schedule(alpha_max: float):
    """Choose (C, L, W): C chunks of length L, W warmup steps."""
    # error after W warmup steps <= alpha_max^W ; want < 1e-9
    if alpha_max >= 1.0 - 1e-9:
        return 1, T, 0
    w_req = int(math.ceil(math.log(1e-9) / math.log(alpha_max)))
    w_req = max(w_req, 4)
    for C in (32, 16, 8, 4, 2):
        L = T // C
        if w_req <= L:
            return C, L, min(max(w_req, 12), L)
    return 1, T, 0


def _build(C: int, L: int, W: int, nt: int, groups: int, alpha_const: float | None = None):
    """Build the per-core Bass program (SPMD: same program on all cores)."""
    nc = bacc.Bacc(target_bir_lowering=False)

    xt = nc.dram_tensor("xt", [D, T, BC], F32R, kind="ExternalInput")
    wT = nc.dram_tensor("wT", [D, D], F32R, kind="ExternalInput")
    alpha_rec = nc.dram_tensor("alpha_rec", [128, EO * C * BC], F32, kind="ExternalInput")
    b_ev = nc.dram_tensor("b_ev", [128, EO], F32, kind="ExternalInput")
    bg_ev = nc.dram_tensor("bg_ev", [128, EO], F32, kind="ExternalInput")
    h0t = nc.dram_tensor("h0t", [128, EO * BC], F32, kind="ExternalInput")
    outT = nc.dram_tensor("outT", [L, 128, EO, C, BC], F32, kind="ExternalOutput")
    hT = nc.dram_tensor("hT", [L, 128, EO, C, BC], F32, kind="ExternalOutput")

    TP = T + W                # padded t-extent of the xp buffer (front pad W)
    NB = T // nt              # matmul n-blocks (nt t-values each)
    N = nt * BC               # moving cols per matmul
    steps = W + L
    eg = EO // groups         # e_o blocks per group
    gw = eg * C * BC          # per-group state width

    with TileContext(nc) as tc:
        with (
            tc.tile_pool(name="big", bufs=1) as big,
            tc.tile_pool(name="stream", bufs=2) as stream,
            tc.tile_pool(name="small", bufs=1) as small,
            tc.tile_pool(name="state", bufs=1) as statep,
            tc.tile_pool(name="og", bufs=3) as ogp,
            tc.tile_pool(name="psum", bufs=8, space="PSUM") as psum,
        ):
            # resident buffers
            xp_sb = big.tile([128, EO * TP * BC], F32, tag="xp")
            gate_sb = big.tile([128, EO * T * BC], F32, tag="gate")
            w_sb = [big.tile([128, D], F32R, tag=f"w{k}", name=f"w{k}") for k in range(EO)]
            al_sb = small.tile([128, EO * C * BC], F32, tag="alpha")
            bev_sb = small.tile([128, EO], F32, tag="bev")
            bgev_sb = small.tile([128, EO], F32, tag="bgev")
            h0_sb = small.tile([128, EO * BC], F32, tag="h0")

            nc.sync.dma_start(out=bev_sb, in_=b_ev[:, :])
            nc.sync.dma_start(out=bgev_sb, in_=bg_ev[:, :])
            nc.sync.dma_start(out=al_sb, in_=alpha_rec[:, :])
            nc.sync.dma_start(out=h0_sb, in_=h0t[:, :])

            xp4 = xp_sb.rearrange("p (e t b) -> p e t b", e=EO, b=BC)
            gate4 = gate_sb.rearrange("p (e t b) -> p e t b", e=EO, b=BC)

            # zero the warmup pad region [0, W) of xp
            if W > 0:
                nc.vector.memset(xp4[:, :, 0:W, :], 0.0)

            # ---------------- phase 1: matmul + gate ----------------
            for n in range(NB):
                t0 = n * nt
                xtiles = []
                for k in range(EO):
                    if n == 0:
                        # interleave weight preload with the first n-block's
                        # x tiles so MM k can start as soon as pair k landed
                        nc.sync.dma_start(out=w_sb[k], in_=wT[k * 128:(k + 1) * 128, :])
                    xt_t = stream.tile([128, N], F32R, tag=f"xt{k}")
                    nc.sync.dma_start(
                        out=xt_t.rearrange("p (t b) -> p t b", b=BC),
                        in_=xt[k * 128:(k + 1) * 128, t0:t0 + nt, :],
                    )
                    xtiles.append(xt_t)
                    # gate = silu(x + b_gate) in the same transposed layout
                    nc.scalar.activation(
                        gate4[:, k, t0:t0 + nt, :].rearrange("p t b -> p (t b)"),
                        xt_t.bitcast(F32),
                        AF.Silu,
                        bias=bgev_sb[:, k:k + 1],
                    )
                for em in range(EO):
                    ps = psum.tile([128, N], F32, tag="ps")
                    for k in range(EO):
                        nc.tensor.matmul(
                            ps,
                            w_sb[k][:, em * 128:(em + 1) * 128],
                            xtiles[k],
                            start=(k == 0),
                            stop=(k == EO - 1),
                        )
                    # evict with +b bias into the padded xp buffer
                    nc.scalar.activation(
                        xp4[:, em, W + t0:W + t0 + nt, :].rearrange("p t b -> p (t b)"),
                        ps,
                        AF.Identity,
                        bias=bev_sb[:, em:em + 1],
                    )

            # ---------------- phase 2: recurrence ----------------
            # state layout: [128, (e_o, c, b)]; group-split over e_o.
            # h is double-buffered so the hT DMA read never blocks the next
            # step's tanh write.
            h_t = [[statep.tile([128, gw], F32, tag=f"h{g}_{i}", name=f"h{g}_{i}") for i in range(2)]
                   for g in range(groups)]
            u_t = [statep.tile([128, gw], F32, tag=f"u{g}", name=f"u{g}") for g in range(groups)]

            for g in range(groups):
                nc.vector.memset(h_t[g][0], 0.0)
                nc.vector.memset(h_t[g][1], 0.0)

            al3 = al_sb.rearrange("p (e c b) -> p e c b", e=EO, b=BC)
            h03 = h0_sb.rearrange("p (e b) -> p e b", e=EO)

            def gath(buf4, s, g):
                # strided chunk-gather: t = c*L + s for c in [0, C)
                v = buf4[:, :, s::L, :]
                return v[:, g * eg:(g + 1) * eg, 0:C, :]

            for s in range(steps):
                hp = h_t_idx = (s + 1) % 2   # h_prev buffer index
                hn = s % 2                   # h_new buffer index
                if s == W:
                    # inject exact h0 into chunk 0 of every group (h_prev)
                    for g in range(groups):
                        hv = h_t[g][hp].rearrange(
                            "p (e c b) -> p e c b", e=eg, b=BC
                        )
                        nc.vector.tensor_copy(
                            hv[:, :, 0, :], h03[:, g * eg:(g + 1) * eg, :]
                        )
                for g in range(groups):
                    u3 = u_t[g].rearrange("p (e c b) -> p e c b", e=eg, b=BC)
                    h3p = h_t[g][hp].rearrange("p (e c b) -> p e c b", e=eg, b=BC)
                    if alpha_const is not None:
                        # tensor_scalar runs in 2x mode for fp32 (single-src)
                        nc.vector.tensor_scalar_mul(u_t[g], h_t[g][hp], alpha_const)
                    else:
                        av = al3[:, g * eg:(g + 1) * eg, :, :]
                        nc.vector.tensor_mul(u3, h3p, av)
                    # u += xp[t(s)] ; h = tanh(u)
                    nc.vector.tensor_add(u3, u3, gath(xp4, s, g))
                    nc.scalar.activation(h_t[g][hn], u_t[g], AF.Tanh)
                if s >= W:
                    j = s - W
                    for g in range(groups):
                        h3n = h_t[g][hn].rearrange(
                            "p (e c b) -> p e c b", e=eg, b=BC
                        )
                        og = ogp.tile([128, gw], F32, tag=f"og{g}")
                        og3 = og.rearrange("p (e c b) -> p e c b", e=eg, b=BC)
                        # balance the output mult across DVE and GpSimd
                        eng = nc.vector if g % 2 == 0 else nc.gpsimd
                        eng.tensor_mul(og3, h3n, gath(gate4, j, g))
                        nc.sync.dma_start(
                            out=outT[j, :, g * eg:(g + 1) * eg, :, :],
                            in_=og3,
                        )
                        nc.sync.dma_start(
                            out=hT[j, :, g * eg:(g + 1) * eg, :, :],
                            in_=h3n,
                        )

    nc.finalize()
    return nc


_cache = {}


def _get_program(C, L, W, nt, groups, alpha_const):
    key = (C, L, W, nt, groups, alpha_const)
    if key not in _cache:
        _cache[key] = _build(C, L, W, nt, groups, alpha_const)
    return _cache[key]


def kernel(x, h0, W_x, alpha_raw, b, b_gate, _trace=False, _tmpdir=None):
    x = np.asarray(x, dtype=np.float32)
    h0 = np.asarray(h0, dtype=np.float32)
    W_x = np.asarray(W_x, dtype=np.float32)
    alpha_raw = np.asarray(alpha_raw, dtype=np.float32)
    b = np.asarray(b, dtype=np.float32)
    b_gate = np.asarray(b_gate, dtype=np.float32)

    alpha = (1.0 / (1.0 + np.exp(-alpha_raw.astype(np.float64)))).astype(np.float32)
    C, L, W = _pick_schedule(float(alpha.max()))
    nt = int(os.environ.get("K_NT", "128"))
    groups = int(os.environ.get("K_GROUPS", "2"))
    # fast path: alpha constant across D -> fused scalar_tensor_tensor
    alpha_const = None
    if np.all(alpha == alpha[0]) and os.environ.get("K_NO_ACONST") != "1":
        alpha_const = float(alpha[0])

    nc = _get_program(C, L, W, nt, groups, alpha_const)

    # host-side prep (not in HW time)
    xT = np.ascontiguousarray(x.transpose(2, 0, 1))          # [D, T, B]
    wT = np.ascontiguousarray(W_x.T)                          # [d, e]
    al_eo = np.ascontiguousarray(alpha.reshape(EO, 128).T)    # [128, EO]
    alpha_rec = np.ascontiguousarray(
        np.broadcast_to(al_eo[:, :, None, None], (128, EO, C, BC))
    ).reshape(128, EO * C * BC)
    b_ev = np.ascontiguousarray(b.reshape(EO, 128).T)
    bg_ev = np.ascontiguousarray(b_gate.reshape(EO, 128).T)

    in_maps = []
    for c8 in range(NCORES):
        bsl = slice(c8 * BC, (c8 + 1) * BC)
        h0c = h0[bsl, :]                                      # [BC, D]
        h0t = np.ascontiguousarray(
            h0c.T.reshape(EO, 128, BC).transpose(1, 0, 2)
        ).reshape(128, EO * BC)
        in_maps.append({
            "xt": np.ascontiguousarray(xT[:, :, bsl]),
            "wT": wT,
            "alpha_rec": alpha_rec,
            "b_ev": b_ev,
            "bg_ev": bg_ev,
            "h0t": h0t,
        })

    res = run_bass_kernel_spmd(
        nc, in_maps, core_ids=list(range(NCORES)),
        trace=_trace, tmpdir=_tmpdir,
    )

    out = np.empty((T, B, D), dtype=np.float32)
    h = np.empty((T + 1, B, D), dtype=np.float32)
    h[0] = h0
    for c8 in range(NCORES):
        bsl = slice(c8 * BC, (c8 + 1) * BC)
        r = res.results[c8]
        # [L(j), e_i, e_o, c, b] -> (c, j, b, e_o, e_i) -> [T, BC, D]
        out[:, bsl, :] = r["outT"].transpose(3, 0, 4, 2, 1).reshape(T, BC, D)
        h[1:, bsl, :] = r["hT"].transpose(3, 0, 4, 2, 1).reshape(T, BC, D)
    if _trace:
        kernel._last_result = res
    return out, h
